# revision 25
# baseline (speedup 1.0000x reference)
"""
Trainium2 distributed kernel for causal multi-head attention
(nn_Attention: B=4, S=2048, D=768, H=4, DH=192).

Sharding: 16 (batch, head) units across 8 cores = 1 batch x 2 heads per
core.  Every core runs an identical graph (SPMD) on its own shard; the
host sums core pairs (the unshard for output-partial sharding).  No
on-device collectives, perfectly balanced causal work.

Device algorithm (bf16 matmuls, f32 PSUM accumulation):
  QT/KT stored transposed [head-dim planes, seq]; the two heads' upper
  64 head-dims share one 128-partition plane (host permutes weight
  columns to match), so every projection matmul contracts a full 128
  partitions and the two 64-row score matmuls run in disjoint PE row
  groups (concurrent).  V is stored naturally [seq, dh] with per-head
  ones columns so the AV matmul also emits softmax denominators.
  Scores are computed transposed, S.T[k, q] = KT.T @ QT, so exp'd
  attention tiles feed AV directly as the moving operand — no
  transposes anywhere.  Softmax skips max-subtraction (logits are O(1)
  by construction); causality is applied post-exp as a multiplicative
  0/1 bf16 mask on the 128-wide diagonal strips only; diagonal blocks
  restrict the moving-q range to the live columns (both in scores and
  AV), with h1's half left-aligned so the psum written region stays
  contiguous.  Outputs are written bf16 (host sums partials in fp32).
  The emission schedule processes big q-blocks first with AV deferred
  one block (the next block's scores cover the exp tail).  Within each
  AV, the denominator (c1) matmul chains run before the c0 chains, so
  the softmax-norm pipeline hides under the c0 matmuls and only h1's
  two rescale muls remain after AV ends.

  Softmax reciprocals run on ACT as exp(-ln(x)) — the denominator row
  is a single partition, so the DVE iterative-divide reciprocal
  (8 cyc/elem through one lane, ~3.3us) would sit on the critical
  path; a manually preloaded combined ln+exp activation-table set
  (natural_log_exp_and_others) makes both functions resident with one
  table load, no thrash against the score exps.  The last AV (qj=0)
  is sandwiched between the out_proj(2) halves so its recip/broadcast/
  rescale chain resolves under those matmuls and the final
  out-projections run back-to-back on a warm clock.  Input DMAs are
  ordered by first use across two queues (sync: wqk-Q + xT planes 0-2;
  gpsimd: wqk-K planes + xT 3-5) with wv/wo last so they don't steal
  HBM bandwidth from the lead-in; a ~2.8us warmup spin of dummy
  matmuls trips the HAM clock-gate (1.2 -> 2.4 GHz) during the DMA
  lead-in.  Projection psum-evacuation copies split Q->ACT / K->DVE
  (both idle then), and the out-proj tail alternates copy engines
  (ACT/DVE) and DMA queues by (m+n) parity with a quarter-granular
  final block so the drain is two parallel lanes.
"""

import math
import os
import sys

import numpy as np

for _p in ("/opt/trn_rl_repo",):
    if _p not in sys.path and os.path.isdir(_p):
        sys.path.insert(0, _p)

import ml_dtypes  # noqa: E402

B, S, D, H = 4, 2048, 768, 4
DH = D // H  # 192
HPC = 2  # heads per core
HD = HPC * DH  # 384 head dims per core
P = 128
KD = D // P  # 6 contraction chunks over D
QB = 512  # query block (matmul moving dim)
NQ = S // QB  # 4
KB = 128  # key block (psum partition dim)
NK = S // KB  # 16
MS = S // P  # 16 seq chunks
SCALE = 1.0 / math.sqrt(DH)

# host-side column permutation for Wq/Wk (and row perm for Wo):
# planes = [h0 dh0:128 | h1 dh0:128 | h0 dh128:192, h1 dh128:192]
PQ = np.r_[0:128, 192:320, 128:192, 320:384]
# V sbuf layout is natural per head: [h0 dh 192 | ones0 | h1 dh 192 |
# ones1], so the psum [h0 192 | h1 192] fills with two contiguous
# copies (the second just shifted past the ones0 column)

_CACHED = {}


def build_nc(reps=1, actcopy=False, endact=True, OP1_END=False):
    import concourse.mybir as mybir
    from concourse import bacc
    from concourse import tile

    fp32 = mybir.dt.float32
    bf16 = mybir.dt.bfloat16
    Exp = mybir.ActivationFunctionType.Exp
    Ln = mybir.ActivationFunctionType.Ln

    nc = bacc.Bacc(None, target_bir_lowering=False)

    xT = nc.declare_dram_parameter("xT", [D, S], bf16, isOutput=False)
    # wq and wk packed side by side: [row | wq cols 384 | wk cols 384]
    # so one plane DMA moves 1.5KB rows (efficient) and feeds both
    wqk = nc.declare_dram_parameter("wqk", [D, 2 * HD], bf16, isOutput=False)
    wvT = nc.declare_dram_parameter("wvT", [D, HD], bf16, isOutput=False)
    woS = nc.declare_dram_parameter("woS", [HD, D], bf16, isOutput=False)
    # bf16 partial outputs: the host sums core pairs in fp32, so the only
    # cost is one rounding of each partial (~0.2% rel) — halves out DMA
    out = nc.declare_dram_parameter("out", [S, D], bf16, isOutput=True)

    # V sbuf free-layout offsets
    V_H0C0 = slice(0, 128)
    V_H0C1 = slice(128, 193)  # h0 dh128:192 + ones0 @192 -> denom row 64
    V_H1C0 = slice(193, 321)
    V_H1C1 = slice(321, 386)  # h1 dh128:192 + ones1 @385 -> denom row 64
    VW = 386

    with tile.TileContext(nc) as tc:
        with (
            tc.tile_pool(name="const", bufs=1) as const,
            tc.tile_pool(name="atp", bufs=2) as atp,
            tc.tile_pool(name="ost", bufs=6) as ostp,
            tc.tile_pool(name="rcp", bufs=2) as rcp,
            tc.tile_pool(name="scps", bufs=2, space="PSUM") as scps,
            tc.tile_pool(name="avps", bufs=1, space="PSUM") as avps,
        ):
            # ---- persistent SBUF tensors ----
            xT_sb = const.tile([P, KD, S], bf16, tag="xT_sb")
            wqk_sb = const.tile([P, KD, 2 * HD], bf16, tag="wqk_sb")
            wv_sb = const.tile([P, KD, HD], bf16, tag="wv_sb")
            wo_sb = const.tile([P, 3, D], bf16, tag="wo_sb")
            qt_sb = const.tile([P, 3, S], bf16, tag="qt_sb")
            kt_sb = const.tile([P, 3, S], bf16, tag="kt_sb")
            v_sb = const.tile([P, NK, VW], bf16, tag="v_sb")
            pt_sb = const.tile([P, 3, S], bf16, tag="pt_sb")
            ones1 = const.tile([1, P], bf16, tag="ones1")
            ones5 = const.tile([1, QB], bf16, tag="ones5")
            warm = const.tile([1, 1], fp32, tag="warm")
            dmask = const.tile([P, KB], bf16, tag="dmask")

            def wq_pl(k, c):
                return wqk_sb[:, k, c * P : (c + 1) * P]

            def wk_pl(k, c):
                return wqk_sb[:, k, HD + c * P : HD + (c + 1) * P]

            # ---- input DMAs ----
            # two queues, ordered by first-use: sync (HWDGE, lowest
            # latency) carries the wqk plane-0 Q columns plus xT planes
            # 0-2; gpsimd carries the wqk K columns + planes 1-5 and xT
            # planes 3-5, in wave-k need order.  wv/wo (1.1MB, not
            # consumed until the V projection ~25us later) go LAST on
            # gpsimd so their descriptors don't steal HBM bandwidth from
            # the lead-in.  Plane 0 loads upper half first (wave k=0
            # runs nt order 2,3,0,1); later planes lower half first.
            HS = S // 2
            def xt_half(q, k, lo):
                sl = slice(0, HS) if lo else slice(HS, S)
                q.dma_start(xT_sb[:, k, sl], xT[k * P : (k + 1) * P, sl])

            nc.sync.dma_start(wqk_sb[:, 0, 0:HD], wqk[0:P, 0:HD])
            xt_half(nc.sync, 0, False)
            xt_half(nc.sync, 0, True)
            for k in (1, 2):
                xt_half(nc.sync, k, True)
                xt_half(nc.sync, k, False)
            nc.gpsimd.dma_start(wqk_sb[:, 0, HD : 2 * HD], wqk[0:P, HD:])
            nc.gpsimd.dma_start(wqk_sb[:, 1, :], wqk[P : 2 * P, :])
            nc.gpsimd.dma_start(wqk_sb[:, 2, :], wqk[2 * P : 3 * P, :])
            for k in (3, 4, 5):
                nc.gpsimd.dma_start(
                    wqk_sb[:, k, :], wqk[k * P : (k + 1) * P, :]
                )
                xt_half(nc.gpsimd, k, True)
                xt_half(nc.gpsimd, k, False)
            nc.gpsimd.dma_start(
                wv_sb[:], wvT.rearrange("(ko ki) j -> ki ko j", ki=P)
            )
            nc.gpsimd.dma_start(
                wo_sb[:], woS.rearrange("(ko ki) j -> ki ko j", ki=P)
            )

            nc.vector.memset(ones1[:], 1.0)
            nc.vector.memset(ones5[:], 1.0)
            # preload the combined ln+exp table set (index 6 =
            # natural_log_exp_and_others) so the softmax reciprocals can
            # run as exp(-ln(x)) on ACT without table thrash — the
            # automatic pass would pick exp_and_others for Exp and
            # natural_log for Ln and reload (~2.7us) on every switch.
            nc.scalar.add_instruction(
                mybir.InstLoadActFuncSet(
                    name=nc.get_next_instruction_name(), act_func_set_id=6
                )
            )
            # prefetch the exp path while the PE does projections
            nc.scalar.activation(warm[:], ones1[0:1, 0:1], Exp)
            # ones columns of V are static: set them once
            nc.vector.memset(v_sb[:, :, 192:193], 1.0)
            nc.vector.memset(v_sb[:, :, 385:386], 1.0)

            # single 128x128 triangular mask (0/1 bf16) for the diagonal
            # strip of every causal block: keep 1 iff q_local >= k_local
            nc.vector.memset(dmask[:], 1.0)
            nc.gpsimd.affine_select(
                out=dmask[:],
                in_=dmask[:],
                compare_op=mybir.AluOpType.is_ge,
                fill=0.0,
                base=0,
                pattern=[[1, KB]],
                channel_multiplier=-1,
            )

            def pcopy(dst, src, on_act=actcopy):
                (nc.scalar.copy if on_act else nc.vector.tensor_copy)(
                    dst, src
                )

            # PE clock-ramp warmup: dummy matmuls on ones burn down the
            # HAM p-state window while the first input DMAs are in
            # flight.  The window needs ~3.4us of sustained PE activity
            # to un-throttle (K=4/8 -> 8/8), and the input lead-in is
            # ~3us, so spin six N=512 + four N=64 matmuls (~2.8us cold)
            # back to back — the real projection stream then starts at
            # full clock instead of warming up ~15us in.
            wps = avps.tile([P, QB], fp32, tag="avA", name="warm_ps")
            for _ in range(6):
                nc.tensor.matmul(
                    wps[0:1, :],
                    lhsT=ones1[0:1, 0:1],
                    rhs=ones5[0:1, :],
                    start=True, stop=True,
                )
            for _ in range(4):
                nc.tensor.matmul(
                    wps[0:1, 0:64],
                    lhsT=ones1[0:1, 0:1],
                    rhs=ones1[:, 0:64],
                    start=True, stop=True,
                )

            # ---- Q/K projections (transposed outputs, 3 full planes) ----
            def wide_wave():
                # per xT plane: Q-c0's 4 groups (av tags) + K-c0's 4
                # groups (2 double-width sc slots) -> 8 matmuls per plane
                # arrival; nt order starts on the upper half (first DMA in)
                pssQ = [
                    avps.tile([P, QB], fp32, tag=t, name=f"wwq_{t}")
                    for t in ("avA", "avB", "avC", "avD")
                ]
                kts = [
                    scps.tile([P, 2 * QB], fp32, tag="sc", name=f"wwk_{i}")
                    for i in range(2)
                ]
                pssK = [
                    kts[0][:, 0:QB],
                    kts[0][:, QB : 2 * QB],
                    kts[1][:, 0:QB],
                    kts[1][:, QB : 2 * QB],
                ]
                for k in range(KD):
                    order = (2, 3, 0, 1) if k == 0 else (0, 1, 2, 3)
                    for nt in order:
                        nc.tensor.matmul(
                            pssQ[nt],
                            lhsT=wq_pl(k, 0),
                            rhs=xT_sb[:, k, nt * QB : (nt + 1) * QB],
                            start=(k == 0), stop=(k == KD - 1),
                        )
                    for nt in order:
                        nc.tensor.matmul(
                            pssK[nt],
                            lhsT=wk_pl(k, 0),
                            rhs=xT_sb[:, k, nt * QB : (nt + 1) * QB],
                            start=(k == 0), stop=(k == KD - 1),
                        )

                # Q copies on ACT (idle until the first scores' exps),
                # K copies on DVE — the two drain concurrently
                for nt in range(NQ):
                    pcopy(qt_sb[:, 0, nt * QB : (nt + 1) * QB], pssQ[nt], True)
                for nt in range(NQ):
                    pcopy(kt_sb[:, 0, nt * QB : (nt + 1) * QB], pssK[nt], False)

            def projections(first=False):
                if first:
                    wide_wave()
                for w_pl, o_sb in ((wq_pl, qt_sb), (wk_pl, kt_sb)):
                    for c in range(1 if first else 0, 3):
                        for nt in range(NQ):
                            ps = avps.tile(
                                [P, QB], fp32,
                                tag="av" + "ABCD"[nt], name=f"pj{c}{nt}",
                            )
                            for k in range(KD):
                                nc.tensor.matmul(
                                    ps,
                                    lhsT=w_pl(k, c),
                                    rhs=xT_sb[:, k, nt * QB : (nt + 1) * QB],
                                    start=(k == 0),
                                    stop=(k == KD - 1),
                                )
                            pcopy(
                                o_sb[:, c, nt * QB : (nt + 1) * QB], ps,
                                o_sb is qt_sb,
                            )
            # ---- V projection (natural layout) + ones columns ----
            # emitted separately, after the a3 scores: V isn't consumed
            # until av3, and the scheduler interleaves these matmuls
            # (avps pool) into the exp-paced score stream (sc pool), so
            # the ACT exp backlog drains with zero PE stalls
            def v_projection():
                for m in range(MS):
                    ps = avps.tile(
                        [P, QB], fp32, tag="av" + "ABCD"[m % 4], name=f"pv{m}"
                    )
                    for k in range(KD):
                        nc.tensor.matmul(
                            ps[:, 0:HD],
                            lhsT=xT_sb[:, k, m * P : (m + 1) * P],
                            rhs=wv_sb[:, k, :],
                            start=(k == 0),
                            stop=(k == KD - 1),
                        )
                    pcopy(v_sb[:, m, 0:192], ps[:, 0:192])
                    nc.vector.tensor_copy(v_sb[:, m, 193:385], ps[:, 192:384])

            # ---- attention phases (emitted piecewise by the scheduler) ----
            def out_proj(qj, mis=(0, 1, 2, 3), on_act=False, inj=None,
                         end=False, last=False, act_both=False):
                for mi in mis:
                    m = qj * 4 + mi
                    fine = last and mi == mis[-1]
                    ost = ostp.tile([P, D], bf16, tag="ost")
                    for n in range(2):
                        # the tail out-projs ride the (now free) av psum
                        # tags: a 4-deep ring that hides the copy WAR lag
                        ps = (
                            avps.tile(
                                [P, QB], fp32,
                                tag="av" + "ABCD"[(2 * mi + n) % 4],
                                name=f"ope{qj}{mi}{n}",
                            )
                            if end
                            else scps.tile(
                                [P, QB], fp32, tag="sc", name=f"op{mi}{n}"
                            )
                        )
                        for c in range(3):
                            nc.tensor.matmul(
                                ps[:, 0:384],
                                lhsT=pt_sb[:, c, m * P : (m + 1) * P],
                                rhs=wo_sb[:, c, n * 384 : (n + 1) * 384],
                                start=(c == 0),
                                stop=(c == 2),
                            )
                        # alternate ACT/DVE so consecutive copies run
                        # concurrently instead of serializing on one engine;
                        # the very last block goes quarter-granular so its
                        # final DMA chains off a 192-col copy, not a 384
                        if fine:
                            for sub in range(2):
                                c0 = n * 384 + sub * 192
                                pcopy(
                                    ost[:, c0 : c0 + 192],
                                    ps[:, sub * 192 : (sub + 1) * 192],
                                    sub == 0,
                                )
                                [nc.sync, nc.gpsimd][sub].dma_start(
                                    out[m * P : (m + 1) * P, c0 : c0 + 192],
                                    ost[:, c0 : c0 + 192],
                                )
                        else:
                            # (m+n) parity so consecutive m-blocks use
                            # opposite engine/queue lanes at the tail
                            lane = (m + n) % 2 if end else n
                            pcopy(
                                ost[:, n * 384 : (n + 1) * 384],
                                ps[:, 0:384],
                                act_both or lane == 0,
                            )
                            [nc.sync, nc.gpsimd][lane].dma_start(
                                out[m * P : (m + 1) * P,
                                    n * 384 : (n + 1) * 384],
                                ost[:, n * 384 : (n + 1) * 384],
                            )
                    if inj is not None and mi in inj:
                        inj[mi]()

            def scores_mms(qj, inj=None):
                nk = 4 * qj + 4  # live key blocks (causal)
                # fused at tile: both heads side by side [.., h0 512|h1 512]
                at2 = atp.tile(
                    [P, NK, 2 * QB], bf16, tag="at2", name=f"at2_{qj}"
                )
                for ki in range(nk):
                    ksl = slice(ki * KB, (ki + 1) * KB)
                    d = ki - 4 * qj
                    # diagonal blocks: only q_local >= 128*d attends to
                    # this key block - restrict the moving operand range
                    off = max(d, 0) * KB
                    qsl = slice(qj * QB + off, (qj + 1) * QB)
                    # one 2-bank psum tile holds both heads' score block.
                    # h1 is left-aligned within its half so the written
                    # region [off : 2QB-off] stays contiguous
                    ps = scps.tile([P, 2 * QB], fp32, tag="sc")
                    ps0 = ps[:, off:QB]
                    ps1 = ps[:, QB : 2 * QB - off]
                    # full-plane matmuls (128 contraction rows)
                    nc.tensor.matmul(
                        ps0, lhsT=kt_sb[:, 0, ksl], rhs=qt_sb[:, 0, qsl],
                        start=True, stop=False,
                    )
                    nc.tensor.matmul(
                        ps1, lhsT=kt_sb[:, 1, ksl], rhs=qt_sb[:, 1, qsl],
                        start=True, stop=False,
                    )
                    # 64-row tails in disjoint row groups (concurrent)
                    nc.tensor.matmul(
                        ps0, lhsT=kt_sb[0:64, 2, ksl], rhs=qt_sb[0:64, 2, qsl],
                        start=False, stop=True,
                    )
                    nc.tensor.matmul(
                        ps1,
                        lhsT=kt_sb[64:128, 2, ksl],
                        rhs=qt_sb[64:128, 2, qsl],
                        start=False, stop=True,
                    )
                    # one exp for both heads: amortizes the ACT ramp
                    nc.scalar.activation(
                        at2[:, ki, off : 2 * QB - off],
                        ps[:, off : 2 * QB - off],
                        Exp, scale=SCALE,
                    )
                    if d >= 0:
                        # multiplicative causal zeroing post-exp on the
                        # 128-wide diagonal strip of each head (DVE fast
                        # mode); h1's strip sits left-aligned at QB
                        nc.vector.tensor_mul(
                            at2[:, ki, off : off + KB],
                            at2[:, ki, off : off + KB],
                            dmask[:],
                        )
                        nc.vector.tensor_mul(
                            at2[:, ki, QB : QB + KB],
                            at2[:, ki, QB : QB + KB],
                            dmask[:],
                        )
                    if inj is not None and ki in inj:
                        inj[ki]()
                return at2

            def av_mms(qj, at2):
                # c1 (denominator) phases run FIRST so the softmax-norm
                # chain (recip -> broadcast -> materialize) overlaps the c0
                # phases - by AV end only the pt muls remain outstanding
                nk = 4 * qj + 4
                at0 = at2[:, :, 0:QB]
                at1 = at2[:, :, QB : 2 * QB]

                def asl_of(h, off):
                    # h1's at strip is left-aligned (see scores_mms)
                    return slice(off, QB) if h == 0 else slice(0, QB - off)

                def c1_phase(h, at, psc1, c1sl):
                    for ki in range(nk):
                        off = max(ki - 4 * qj, 0) * KB
                        nc.tensor.matmul(
                            psc1[0:65, off:QB],
                            lhsT=v_sb[:, ki, c1sl],
                            rhs=at[:, ki, asl_of(h, off)],
                            start=(ki == 0), stop=(ki == nk - 1),
                        )

                def c0_phase(h, at, psc0, c0sl):
                    for ki in range(nk):
                        off = max(ki - 4 * qj, 0) * KB
                        nc.tensor.matmul(
                            psc0[:, off:QB],
                            lhsT=v_sb[:, ki, c0sl],
                            rhs=at[:, ki, asl_of(h, off)],
                            start=(ki == 0), stop=(ki == nk - 1),
                        )

                def recip(psc1):
                    # 1/x = exp(-ln x) on ACT (both fns in the preloaded
                    # combined table set).  The DVE reciprocal is an
                    # iterative divide at 8 cyc/elem, and the denominator
                    # row is a single partition — 512 elems through one
                    # lane = ~3.3us serial, squarely on the softmax-norm
                    # critical path.  Two ACT passes cost ~1.2us and run
                    # on an engine that is idle at the tail.
                    lnb = rcp.tile([1, QB], fp32, tag="rcL")
                    rc = rcp.tile([1, QB], bf16, tag="rcB")
                    nc.scalar.activation(lnb[:], psc1[64:65, :], Ln)
                    nc.scalar.activation(rc[:], lnb[:], Exp, scale=-1.0)
                    return rc

                psC = avps.tile([P, QB], fp32, tag="avA", name=f"av1_{qj}0")
                psD = avps.tile([P, QB], fp32, tag="avB", name=f"av1_{qj}1")
                psA = avps.tile([P, QB], fp32, tag="avC", name=f"av0_{qj}0")
                psB = avps.tile([P, QB], fp32, tag="avD", name=f"av0_{qj}1")
                qsl = slice(qj * QB, (qj + 1) * QB)

                c1_phase(0, at0, psC, V_H0C1)
                rc0 = recip(psC)  # DVE, runs under c1-h1
                c1_phase(1, at1, psD, V_H1C1)
                rc1 = recip(psD)  # DVE, runs under c0-h0
                # gpsimd broadcasts the reciprocals across partitions -
                # no PE matmul, no PSUM bank, runs under the c0 phases
                rb0 = rcp.tile([P, QB], bf16, tag="rcb")
                rb1 = rcp.tile([P, QB], bf16, tag="rcb")
                nc.gpsimd.partition_broadcast(rb0[:], rc0[:])
                nc.gpsimd.partition_broadcast(rb1[:], rc1[:])
                c0_phase(0, at0, psA, V_H0C0)
                # h0's muls only need psA/psC + rb0: run under c0-h1
                nc.vector.tensor_mul(
                    pt_sb[0:64, 2, qsl], psC[0:64], rb0[0:64]
                )
                nc.vector.tensor_mul(pt_sb[:, 0, qsl], psA, rb0[:])
                c0_phase(1, at1, psB, V_H1C0)
                return (qj, psB, psD, rb1, qsl)

            def fin(st):
                # the only post-AV normalization: h1's two muls (DVE -
                # gpsimd cannot read PSUM).  psB's release first: the next
                # AV's c0-h1 WAR-waits on it
                qj, psB, psD, rcb1, qsl = st
                nc.vector.tensor_mul(pt_sb[:, 1, qsl], psB, rcb1[:])
                nc.vector.tensor_mul(
                    pt_sb[64:128, 2, qsl], psD[0:64], rcb1[64:128]
                )

            # ---- emission schedule ----
            # big q-blocks first; AV deferred one block so the next block's
            # scores cover its exp tail; out-proj threads the gaps and the
            # last two q-blocks' out-projs form the tail over the pt muls
            for _rep in range(reps):
                projections(first=(_rep == 0))
                a3 = scores_mms(3)
                v_projection()
                a2 = scores_mms(2)
                p3 = av_mms(3, a3)
                a1 = scores_mms(1)
                fin(p3)
                out_proj(3, (0, 1))
                p2 = av_mms(2, a2)
                a0 = scores_mms(0)
                fin(p2)
                out_proj(3, (2, 3))
                out_proj(2, (0, 1))
                p1 = av_mms(1, a1)
                fin(p1)
                # av_mms(0) is sandwiched between the out_proj(2) halves:
                # its softmax-norm chain (ACT recips -> gpsimd broadcasts
                # -> DVE muls) then resolves UNDER the op2/op1 matmul
                # stream instead of stalling the PE for ~5us at the very
                # end (which also re-throttled the clock for the final
                # out-proj blocks).
                out_proj(2, (2,), act_both=True)
                p0 = av_mms(0, a0)
                out_proj(2, (3,), act_both=True)
                fin(p0)
                # hybrid psum tags: m0/m1 ride the free sc slots, m2/m3 the
                # avps ring once fin(p0) releases it - kills the one
                # remaining sc-ring WAR seam
                out_proj(1, (0, 1), on_act=endact, end=False)
                out_proj(1, (2, 3), on_act=endact, end=True)
                out_proj(0, on_act=endact, end=True, last=True)

    nc.compile()
    return nc


def _shard_inputs(x, Wq, Wk, Wv, Wo):
    bf = ml_dtypes.bfloat16
    in_maps = []
    for core in range(8):
        b, hp = core // 2, core % 2
        cols = slice(hp * HD, (hp + 1) * HD)
        wq = Wq[cols, :].T[:, PQ]
        wk = Wk[cols, :].T[:, PQ]
        in_maps.append(
            {
                "xT": np.ascontiguousarray(x[b].T).astype(bf),
                "wqk": np.ascontiguousarray(
                    np.concatenate([wq, wk], axis=1)
                ).astype(bf),
                "wvT": np.ascontiguousarray(Wv[cols, :].T).astype(bf),
                "woS": np.ascontiguousarray(Wo[:, cols].T[PQ, :]).astype(bf),
            }
        )
    return in_maps


def _run(inputs, trace=False, **kw):
    from concourse.bass_utils import run_bass_kernel_spmd

    if "nc" not in _CACHED:
        _CACHED["nc"] = build_nc()
    nc = _CACHED["nc"]
    in_maps = _shard_inputs(
        np.asarray(inputs["x"], np.float32),
        np.asarray(inputs["Wq"], np.float32),
        np.asarray(inputs["Wk"], np.float32),
        np.asarray(inputs["Wv"], np.float32),
        np.asarray(inputs["Wo"], np.float32),
    )
    res = run_bass_kernel_spmd(
        nc, in_maps, core_ids=list(range(8)), trace=trace, **kw
    )
    parts = [np.asarray(r["out"]).astype(np.float32) for r in res.results]
    full = np.empty((B, S, D), np.float32)
    for b in range(B):
        full[b] = parts[2 * b] + parts[2 * b + 1]
    return full, res


def kernel(**inputs) -> np.ndarray:
    full, _ = _run(inputs, trace=False)
    return full

